# revision 14
# baseline (speedup 1.0000x reference)
"""GAT (GATConv + edge scoring) Trainium2 Bass kernel, 8-core SPMD.

Strategy (edge-parallel, dst-bucketed), v3:
  - Nodes sharded by range across 8 cores (6250 each). Edges routed to the
    core owning their dst node, sorted by (src-half, dst-block), padded so all
    cores share one program structure. Self-loops are NOT in the edge stream;
    they are applied densely per dst-block at finalize time.
  - Phase A (per core): xp' = x_shard @ [W | W@a_src | W@a_dst] -> fp16 table
    shard [6250, 384] (xp 256 | a_src hi/lo 4+4 | pad); AllGather -> full
    [50000, 384] fp16 table. a_dst hi/lo kept in a separate fp16 [6250, 8]
    block table.
  - Phase B: per 2048-edge chunk, dma_gather fp16 xp'[src] rows (768B);
    one-hot slot matrices S (edge-major) and ST (slot-major) built on DVE in
    fp16; a_dst expanded per tile via ST^T matmul; max-free softmax numerator
    and denominator accumulated per 128-node dst block via S^T @ msg matmuls
    in PSUM. When a block's half-1 segment completes, finalize inline:
    add the dense self-loop message, h = leaky_relu(num/den + bias) -> fp16
    h table; AllGather.
  - Phase C (per 4096-edge chunk): dma_gather h[src] edge-major; ONE
    dma_start_transpose per chunk -> ghT channel-major. h[dst]^T expanded
    from the SBUF-resident dst block via matmul (lhsT = h_blk columns,
    rhs = ST). repT = ghT * hdT (DVE). scores^T[3, e] = wk05^T @ repT via
    matmuls accumulating over channel halves. PSUM -> scalar-engine copy ->
    DRAM [3, TOTE].
  - Host: un-permute scores to original edge order, add fc1_b.
"""
import os
import sys
import types

import numpy as np

sys.path.insert(0, "/opt/trn_rl_repo")

_last_results = None

N = 50000
E = 1600000
D = 256
HEADS = 4
HID = 64
NCORE = 8
NPC = N // NCORE            # 6250 nodes per core
SPLIT = 3125                # per-core row split for table part A
SZAP = SPLIT                # part-A rows per core
SZBP = 6250 - SPLIT         # part-B rows per core (3125)
SZA = 8 * SZAP              # 25000 rows, int16-addressable
SZB = 8 * SZBP              # 25000 rows, int16-addressable
SPLIT_BLK = SPLIT // 128    # part-A ends inside block 24
NBLK = (NPC + 127) // 128   # 49 blocks per core
LASTR = NPC - 128 * (NBLK - 1)  # rows in last block (106)
ROW = 384                   # fp16 table row: [xp 256 | asrc_hi 4 | asrc_lo 4 | pad]
P = 128
CHT_B = 16                  # tiles per phase-B gather chunk (2048 edges)
CHT_C = 32                  # tiles per phase-C gather chunk (4096 edges)
CHE_B = CHT_B * P
CHE_C = CHT_C * P


def _preprocess(edge_index):
    """Route/sort/pad edges; build per-core device arrays + shared structure."""
    src = edge_index[0].astype(np.int64)
    dst = edge_index[1].astype(np.int64)
    orig = np.arange(E, dtype=np.int64)

    core = dst // NPC
    dl = dst % NPC
    blk = dl // P
    slot = dl % P
    sc_core = src // NPC
    loc = src % NPC
    half = (loc >= SPLIT).astype(np.int64)
    i16 = np.where(loc < SPLIT, sc_core * SZAP + loc,
                   sc_core * SZBP + (loc - SPLIT)).astype(np.int16)

    # counts per (core, half, block)
    key = (core * 2 + half) * NBLK + blk
    cnt = np.bincount(key, minlength=NCORE * 2 * NBLK).reshape(NCORE, 2, NBLK)
    tiles_hb = np.maximum(1, -(-cnt.max(axis=0) // P))  # [2, NBLK]

    # tile layout per half: blocks 0..NBLK-1 then trailing pads (block NBLK-1)
    tile_blocks = []
    tile_half = []
    pos0 = np.zeros((2, NBLK), dtype=np.int64)
    t = 0
    for h in range(2):
        for b in range(NBLK):
            pos0[h, b] = t
            nt = int(tiles_hb[h, b])
            tile_blocks += [b] * nt
            tile_half += [h] * nt
            t += nt
        # pad half run to chunk multiple; glue pads to last block's segment
        pad = (-t) % CHT_C
        tile_blocks += [NBLK - 1] * pad
        tile_half += [h] * pad
        t += pad
    T = t
    tile_blocks = np.array(tile_blocks)
    tile_half = np.array(tile_half)
    # psum segments: contiguous same-(half, blk) runs
    segs = []
    s = 0
    for i in range(1, T + 1):
        if i == T or tile_blocks[i] != tile_blocks[s] or tile_half[i] != tile_half[s]:
            segs.append((int(tile_half[s]), int(tile_blocks[s]), s, i - 1))
            s = i
    TOTE = T * P
    chunk_half_b = tile_half[::CHT_B].copy()
    chunk_half_c = tile_half[::CHT_C].copy()

    per_core = []
    for c in range(NCORE):
        m = core == c
        h_c, b_c = half[m], blk[m]
        s_c, i_c, o_c = slot[m], i16[m], orig[m]
        order = np.lexsort((b_c, h_c))
        h_s, b_s = h_c[order], b_c[order]
        # rank within each (h, b) group
        gkey = h_s * NBLK + b_s
        start_of_group = np.r_[True, gkey[1:] != gkey[:-1]]
        gstart = np.flatnonzero(start_of_group)
        grp = np.cumsum(start_of_group) - 1
        rank = np.arange(len(gkey)) - gstart[grp]
        pos = pos0[h_s, b_s] * P + rank

        slot_stream = np.full(TOTE, 999.0, dtype=np.float32)
        i16_stream = np.zeros(TOTE, dtype=np.int16)
        orig_stream = np.full(TOTE, -1, dtype=np.int64)
        slot_stream[pos] = s_c[order].astype(np.float32)
        i16_stream[pos] = i_c[order]
        orig_stream[pos] = o_c[order]

        # wrapped idx layout: [16, TOTE/16] replicated to 128 partitions
        iw = i16_stream.reshape(TOTE // 16, 16).T.copy()
        iw = np.tile(iw, (NCORE, 1))
        slots_col = slot_stream.reshape(T, P).T.copy()  # [128, T]
        per_core.append(dict(idx_w=np.ascontiguousarray(iw),
                             slots=np.ascontiguousarray(slots_col),
                             orig=orig_stream))

    return dict(T=T, TOTE=TOTE,
                chunk_half_b=chunk_half_b, chunk_half_c=chunk_half_c,
                tile_blocks=tile_blocks, tile_half=tile_half, segs=segs,
                per_core=per_core)


def _build(meta):
    import concourse.bacc as bacc
    import concourse.mybir as mybir
    from concourse.tile import TileContext
    from concourse.library_config import mlp

    F32 = mybir.dt.float32
    F32R = mybir.dt.float32r
    F16 = mybir.dt.float16
    I16 = mybir.dt.int16
    U8 = mybir.dt.uint8
    AF = mybir.ActivationFunctionType
    OP = mybir.AluOpType

    T = meta["T"]
    TOTE = meta["TOTE"]
    chunk_half_b = meta["chunk_half_b"]
    chunk_half_c = meta["chunk_half_c"]
    tile_blocks = meta["tile_blocks"]
    segs = meta["segs"]
    NCHUNK_B = T // CHT_B
    NCHUNK_C = T // CHT_C
    # per-tile flags
    seg_start = {}
    seg_stop = {}
    for (h, b, t0, t1) in segs:
        seg_start[t0] = (h, b)
        seg_stop[t1] = (h, b)

    nc = bacc.Bacc(None, target_bir_lowering=False, debug=False,
                   num_devices=NCORE)

    # --- I/O ---
    x_shard = nc.dram_tensor("x_shard", [NPC, D], F32, kind="ExternalInput")
    w0e = nc.dram_tensor("w0e", [P, 264], F32, kind="ExternalInput")
    w1e = nc.dram_tensor("w1e", [P, 264], F32, kind="ExternalInput")
    wk_d = nc.dram_tensor("wk_d", [P, 2, 3], F16, kind="ExternalInput")
    bias_bc = nc.dram_tensor("bias_bc", [P, D], F32, kind="ExternalInput")
    eye_d = nc.dram_tensor("eye_d", [P, P], F32, kind="ExternalInput")
    idx_d = nc.dram_tensor("idx_d", [P, TOTE // 16], I16, kind="ExternalInput")
    slots_d = nc.dram_tensor("slots_d", [P, T], U8, kind="ExternalInput")
    slots_r_d = nc.dram_tensor("slots_r_d", [T, P], U8, kind="ExternalInput")
    iota_row8_d = nc.dram_tensor("iota_row8_d", [P, P], U8, kind="ExternalInput")
    iota_col_d = nc.dram_tensor("iota_col_d", [P, 1], U8, kind="ExternalInput")

    xp_shard = nc.dram_tensor("xp_shard", [NPC, ROW], F16)
    adst_tbl = nc.dram_tensor("adst_tbl", [NPC, 8], F16)
    xp_fullA = nc.dram_tensor("xp_fullA", [SZA, ROW], F16, addr_space="Shared")
    xp_fullB = nc.dram_tensor("xp_fullB", [SZB, ROW], F16, addr_space="Shared")
    h_shard = nc.dram_tensor("h_shard", [NPC, D], F16)
    h_fullA = nc.dram_tensor("h_fullA", [SZA, D], F16, addr_space="Shared")
    h_fullB = nc.dram_tensor("h_fullB", [SZB, D], F16, addr_space="Shared")
    scores_t = nc.dram_tensor("scores_t", [3, TOTE], F32,
                              kind="ExternalOutput")
    debug = os.environ.get("KERNEL_DEBUG", "0") == "1"
    if debug:
        dbg_xp = nc.dram_tensor("dbg_xp", [NPC, ROW], F16,
                                kind="ExternalOutput")
        dbg_h = nc.dram_tensor("dbg_h", [NPC, D], F16, kind="ExternalOutput")

    from contextlib import ExitStack
    with TileContext(nc) as tc, ExitStack() as stk:
        cst = stk.enter_context(tc.tile_pool(name="cst", bufs=1))
        persist = stk.enter_context(tc.tile_pool(name="persist", bufs=1))

        nc.gpsimd.load_library(mlp)
        reg_b = nc.gpsimd.to_reg(CHE_B)
        reg_c = nc.gpsimd.to_reg(CHE_C)

        # constants
        eye_f = cst.tile([P, P], F32)
        nc.sync.dma_start(out=eye_f[:], in_=eye_d.ap())
        w0_t = cst.tile([P, 264], F32)
        nc.sync.dma_start(out=w0_t[:], in_=w0e.ap())
        w0_r = cst.tile([P, 264], F32R)
        nc.vector.tensor_copy(out=w0_r[:], in_=w0_t[:])
        w1_t = cst.tile([P, 264], F32)
        nc.sync.dma_start(out=w1_t[:], in_=w1e.ap())
        w1_r = cst.tile([P, 264], F32R)
        nc.vector.tensor_copy(out=w1_r[:], in_=w1_t[:])
        wk_t = cst.tile([P, 2, 3], F16)
        nc.sync.dma_start(out=wk_t[:], in_=wk_d.ap())
        iota_col = cst.tile([P, 1], U8)
        nc.sync.dma_start(out=iota_col[:], in_=iota_col_d.ap())
        iota_row8 = cst.tile([P, P], U8)
        nc.sync.dma_start(out=iota_row8[:], in_=iota_row8_d.ap())
        bias_t = cst.tile([P, D], F32)
        nc.sync.dma_start(out=bias_t[:], in_=bias_bc.ap())

        # bulk edge metadata (persistent across B and C)
        idx_sb = persist.tile([P, TOTE // 16], I16)
        nc.sync.dma_start(out=idx_sb[:], in_=idx_d.ap())
        slots_sb = persist.tile([P, T], U8)
        nc.sync.dma_start(out=slots_sb[:], in_=slots_d.ap())

        # ---------------- Phase A: xp' table shard ----------------
        with tc.tile_pool(name="pa_sb", bufs=5) as pa, \
             tc.tile_pool(name="pa_ps", bufs=3, space="PSUM") as pa_ps, \
             tc.tile_pool(name="pa_ps2", bufs=3, space="PSUM") as pa_ps2:
            for j in range(NBLK):
                r = P if j < NBLK - 1 else LASTR
                xt = pa.tile([P, D], F32, name=f"xt{j}", tag="xt")
                nc.sync.dma_start(out=xt[:r, :],
                                  in_=x_shard[j * P:j * P + r, :])
                xT = pa.tile([P, 2, P], F32R, name=f"xT{j}", tag="xT")
                for cchunk in range(2):
                    tps = pa_ps.tile([P, P], F32, name=f"tps{j}_{cchunk}",
                                     tag="tps")
                    nc.tensor.transpose(tps[:, :r], in_=xt[:r, cchunk * P:(cchunk + 1) * P],
                                        identity=eye_f[:r, :r])
                    nc.vector.tensor_copy(out=xT[:, cchunk, :r], in_=tps[:, :r])
                pxp = pa_ps2.tile([P, 264], F32, name=f"pxp{j}", tag="pxp")
                nc.tensor.matmul(pxp[:r, :], lhsT=xT[:, 0, :r], rhs=w0_r[:],
                                 start=True, stop=False)
                nc.tensor.matmul(pxp[:r, :], lhsT=xT[:, 1, :r], rhs=w1_r[:],
                                 start=False, stop=True)
                stg = pa.tile([P, 264], F16, name=f"stg{j}", tag="stg")
                # xp + asrc_hi (cols 0:260)
                nc.vector.tensor_copy(out=stg[:r, 0:260], in_=pxp[:r, 0:260])
                # asrc_lo = f32(asrc) - fp16(asrc_hi), rounded to fp16
                nc.vector.tensor_sub(out=stg[:r, 260:264],
                                     in0=pxp[:r, 256:260],
                                     in1=stg[:r, 256:260])
                nc.sync.dma_start(out=xp_shard[j * P:j * P + r, 0:264],
                                  in_=stg[:r, :])
                # adst hi/lo -> [NPC, 8]
                stg2 = pa.tile([P, 8], F16, name=f"stg2_{j}", tag="stg2")
                nc.vector.tensor_copy(out=stg2[:r, 0:4], in_=pxp[:r, 260:264])
                nc.vector.tensor_sub(out=stg2[:r, 4:8],
                                     in0=pxp[:r, 260:264],
                                     in1=stg2[:r, 0:4])
                nc.sync.dma_start(out=adst_tbl[j * P:j * P + r, :],
                                  in_=stg2[:r, :])
                if j == (SPLIT - 1) // P:
                    nc.gpsimd.collective_compute(
                        "AllGather", mybir.AluOpType.bypass,
                        replica_groups=[list(range(NCORE))],
                        ins=[xp_shard[0:SPLIT, :]], outs=[xp_fullA[:]])

        nc.gpsimd.collective_compute(
            "AllGather", mybir.AluOpType.bypass,
            replica_groups=[list(range(NCORE))],
            ins=[xp_shard[SPLIT:NPC, :]], outs=[xp_fullB[:]])

        # ---------------- Phase B: message accumulation ----------------
        with tc.tile_pool(name="pb_g", bufs=4) as pb_g, \
             tc.tile_pool(name="pb_s", bufs=2) as pb_s, \
             tc.tile_pool(name="pb_m", bufs=2) as pb_m, \
             tc.tile_pool(name="pb_sm", bufs=3) as pb_sm, \
             tc.tile_pool(name="pb_partials", bufs=1) as pb_part, \
             tc.tile_pool(name="pb_blk", bufs=2) as pb_blk, \
             tc.tile_pool(name="pb_h", bufs=2) as pb_h, \
             tc.tile_pool(name="pb_ps_ad", bufs=2, space="PSUM") as ps_ad, \
             tc.tile_pool(name="pb_ps_blk", bufs=4, space="PSUM") as ps_blkp:
            partials = pb_part.tile([P, NBLK, 264], F32)
            cur_ps = None
            cur_adst = None
            ps_of_tile = {}

            def finalize_block(b):
                r = P if b < NBLK - 1 else LASTR
                # dense self-loop: node n (slot s of block b) contributes
                # exp(leaky(asrc[n]+adst[n])) * xp[n] to its own num/den.
                xpb = pb_h.tile([P, 264], F16, name=f"fxp{b}", tag="fxp")
                nc.sync.dma_start(out=xpb[:r, :],
                                  in_=xp_shard[b * P:b * P + r, 0:264])
                adb = pb_h.tile([P, 8], F16, name=f"fad{b}", tag="fad")
                nc.sync.dma_start(out=adb[:r, :],
                                  in_=adst_tbl[b * P:b * P + r, :])
                asum = pb_h.tile([P, 4], F32, name=f"fas{b}", tag="fas")
                nc.vector.tensor_tensor(out=asum[:r], in0=xpb[:r, 256:260],
                                        in1=xpb[:r, 260:264], op=OP.add)
                adsum = pb_h.tile([P, 4], F32, name=f"fads{b}", tag="fads")
                nc.vector.tensor_tensor(out=adsum[:r], in0=adb[:r, 0:4],
                                        in1=adb[:r, 4:8], op=OP.add)
                al = pb_h.tile([P, 4], F32, name=f"fal{b}", tag="fal")
                nc.vector.tensor_tensor(out=al[:r], in0=asum[:r],
                                        in1=adsum[:r], op=OP.add)
                al2 = pb_h.tile([P, 4], F32, name=f"fal2{b}", tag="fal2")
                nc.vector.scalar_tensor_tensor(
                    out=al2[:r], in0=al[:r], scalar=0.2, in1=al[:r],
                    op0=OP.mult, op1=OP.max)
                exs = pb_h.tile([P, 4], F32, name=f"fex{b}", tag="fex")
                nc.scalar.activation(exs[:r], al2[:r], AF.Exp)
                den = pb_h.tile([P, 4], F32, name=f"den{b}", tag="den")
                nc.vector.tensor_tensor(out=den[:r], in0=partials[:r, b, 256:260],
                                        in1=exs[:r], op=OP.add)
                rec = pb_h.tile([P, 4], F32, name=f"rec{b}", tag="rec")
                nc.vector.reciprocal(rec[:r], den[:r])
                slm = pb_h.tile([P, D], F32, name=f"slm{b}", tag="slm")
                nc.vector.tensor_tensor(
                    out=slm[:r].rearrange("p (h c) -> p h c", c=HID),
                    in0=xpb[:r, 0:256].rearrange("p (h c) -> p h c", c=HID),
                    in1=exs[:r, :, None].to_broadcast([r, 4, HID]),
                    op=OP.mult)
                num = pb_h.tile([P, D], F32, name=f"num{b}", tag="num")
                nc.vector.tensor_tensor(out=num[:r], in0=partials[:r, b, 0:256],
                                        in1=slm[:r], op=OP.add)
                z = pb_h.tile([P, D], F32, name=f"z{b}", tag="z")
                nc.vector.tensor_tensor(
                    out=z[:r].rearrange("p (h c) -> p h c", c=HID),
                    in0=num[:r].rearrange("p (h c) -> p h c", c=HID),
                    in1=rec[:r, :, None].to_broadcast([r, 4, HID]),
                    op=OP.mult)
                z2 = pb_h.tile([P, D], F32, name=f"z2{b}", tag="z2")
                nc.vector.tensor_add(out=z2[:r], in0=z[:r], in1=bias_t[:r])
                ht = pb_h.tile([P, D], F16, name=f"ht{b}", tag="ht")
                nc.vector.scalar_tensor_tensor(
                    out=ht[:r], in0=z2[:r], scalar=0.01, in1=z2[:r],
                    op0=OP.mult, op1=OP.max)
                nc.sync.dma_start(out=h_shard[b * P:b * P + r, :],
                                  in_=ht[:r, :])

            for ci in range(NCHUNK_B):
                h = int(chunk_half_b[ci])
                g = pb_g.tile([P, CHT_B, ROW], F16, name=f"g{ci}", tag="g")
                xp_src = xp_fullA if h == 0 else xp_fullB
                nc.gpsimd.dma_gather(
                    g[:], xp_src[:, :],
                    idx_sb[:, ci * (CHE_B // 16):(ci + 1) * (CHE_B // 16)],
                    CHE_B, reg_b, ROW, single_packet=(CHE_B <= 1024))
                S_all = pb_s.tile([P, CHT_B, P], F16, name=f"S{ci}", tag="S")
                nc.vector.tensor_tensor(
                    out=S_all[:],
                    in0=slots_sb[:, ci * CHT_B:(ci + 1) * CHT_B, None].to_broadcast([P, CHT_B, P]),
                    in1=iota_row8[:, None, :].to_broadcast([P, CHT_B, P]),
                    op=OP.is_equal)
                sbc = pb_s.tile([P, CHT_B, P], U8, name=f"sbc{ci}", tag="sbc")
                nc.sync.dma_start(
                    out=sbc[:],
                    in_=slots_r_d[ci * CHT_B:(ci + 1) * CHT_B, :].partition_broadcast(P))
                ST_all = pb_s.tile([P, CHT_B, P], F16, name=f"ST{ci}", tag="ST")
                nc.vector.tensor_tensor(
                    out=ST_all[:], in0=sbc[:],
                    in1=iota_col[:, :1, None].to_broadcast([P, CHT_B, P]),
                    op=OP.is_equal)
                ps_a = ps_ad.tile([P, CHT_B * 8], F32, name=f"psa{ci}", tag="psa")
                for t in range(CHT_B):
                    gt = ci * CHT_B + t
                    if gt in seg_start:
                        _, b = seg_start[gt]
                        cur_ps = ps_blkp.tile([P, 264], F32, name=f"psb{gt}",
                                              tag="psb")
                        cur_adst = pb_blk.tile([P, 8], F16, name=f"a0_{gt}",
                                               tag="a0")
                        r = P if b < NBLK - 1 else LASTR
                        if r < P:
                            nc.vector.memset(cur_adst[:], 0.0)
                        nc.sync.dma_start(out=cur_adst[:r, :],
                                          in_=adst_tbl[b * P:b * P + r, :])
                    ps_of_tile[gt] = cur_ps
                    nc.tensor.matmul(ps_a[:, t * 8:(t + 1) * 8],
                                     lhsT=ST_all[:, t, :], rhs=cur_adst[:],
                                     start=True, stop=True)
                # alpha/exp for the whole chunk
                # asum = (asrc_hi + adst_hi) + (asrc_lo + adst_lo)
                ps_a_v = ps_a[:].rearrange("p (t f) -> p t f", f=8)
                hi = pb_sm.tile([P, CHT_B, 4], F32, name=f"hi{ci}", tag="hi")
                nc.vector.tensor_tensor(
                    out=hi[:], in0=g[:, :, 256:260], in1=ps_a_v[:, :, 0:4],
                    op=OP.add)
                lo = pb_sm.tile([P, CHT_B, 4], F32, name=f"lo{ci}", tag="lo")
                nc.vector.tensor_tensor(
                    out=lo[:], in0=g[:, :, 260:264], in1=ps_a_v[:, :, 4:8],
                    op=OP.add)
                asum = pb_sm.tile([P, CHT_B, 4], F32, name=f"as{ci}", tag="as")
                nc.vector.tensor_tensor(out=asum[:], in0=hi[:], in1=lo[:],
                                        op=OP.add)
                alpha = pb_sm.tile([P, CHT_B, 4], F32, name=f"al{ci}", tag="al")
                nc.vector.scalar_tensor_tensor(
                    out=alpha[:], in0=asum[:], scalar=0.2, in1=asum[:],
                    op0=OP.mult, op1=OP.max)
                expv = pb_sm.tile([P, CHT_B, 4], F32, name=f"ex{ci}", tag="ex")
                nc.scalar.activation(expv[:], alpha[:], AF.Exp)
                msg = pb_m.tile([P, CHT_B, 264], F16, name=f"m{ci}", tag="m")
                nc.vector.tensor_tensor(
                    out=msg[:, :, 0:256].rearrange("p t (h c) -> p t h c", c=HID),
                    in0=g[:, :, 0:256].rearrange("p t (h c) -> p t h c", c=HID),
                    in1=expv[:, :, :, None].to_broadcast([P, CHT_B, 4, HID]),
                    op=OP.mult)
                nc.vector.tensor_copy(out=msg[:, :, 256:260], in_=expv[:])
                nc.vector.tensor_sub(out=msg[:, :, 260:264], in0=expv[:],
                                     in1=expv[:])
                for t in range(CHT_B):
                    gt = ci * CHT_B + t
                    st_fl = gt in seg_start
                    sp_fl = gt in seg_stop
                    tile_ps = ps_of_tile[gt]
                    nc.tensor.matmul(tile_ps[:], lhsT=S_all[:, t, :],
                                     rhs=msg[:, t, :],
                                     start=st_fl, stop=sp_fl)
                    if sp_fl:
                        hh, b = seg_stop[gt]
                        if hh == 0:
                            nc.vector.tensor_copy(out=partials[:, b, :],
                                                  in_=tile_ps[:])
                        else:
                            nc.vector.tensor_add(out=partials[:, b, :],
                                                 in0=tile_ps[:],
                                                 in1=partials[:, b, :])
                            finalize_block(b)

        nc.gpsimd.collective_compute(
            "AllGather", mybir.AluOpType.bypass,
            replica_groups=[list(range(NCORE))],
            ins=[h_shard[0:SPLIT, :]], outs=[h_fullA[:]])
        nc.gpsimd.collective_compute(
            "AllGather", mybir.AluOpType.bypass,
            replica_groups=[list(range(NCORE))],
            ins=[h_shard[SPLIT:NPC, :]], outs=[h_fullB[:]])

        # ---------------- Phase C: edge scores (tensor-engine pipeline) ----
        # per-chunk runs of contiguous same-block tiles: (block, t0, t1, new)
        chunk_runs = []
        for ci in range(NCHUNK_C):
            runs = []
            for t in range(CHT_C):
                gt = ci * CHT_C + t
                b = int(tile_blocks[gt])
                if gt in seg_start or t == 0:
                    runs.append([b, t, t + 1, gt in seg_start])
                else:
                    runs[-1][2] = t + 1
            chunk_runs.append(runs)
        NEG = CHT_C // 4  # 4-tile (512-edge) groups per chunk

        with tc.tile_pool(name="pc_g", bufs=4) as pc_g, \
             tc.tile_pool(name="pc_gt", bufs=2) as pc_gt, \
             tc.tile_pool(name="pc_s", bufs=2) as pc_s, \
             tc.tile_pool(name="pc_r", bufs=3) as pc_r, \
             tc.tile_pool(name="pc_blk", bufs=2) as pc_blk, \
             tc.tile_pool(name="pc_st", bufs=2) as pc_st, \
             tc.tile_pool(name="pc_ps_hd", bufs=2, space="PSUM") as ps_hd, \
             tc.tile_pool(name="pc_ps_sc", bufs=3, space="PSUM") as ps_sc:
            cur_hb = None
            for ci in range(NCHUNK_C):
                h = int(chunk_half_c[ci])
                gh = pc_g.tile([P, CHT_C, D], F16, name=f"gh{ci}", tag="gh")
                h_src = h_fullA if h == 0 else h_fullB
                nc.gpsimd.dma_gather(
                    gh[:], h_src[:, :],
                    idx_sb[:, ci * (CHE_C // 16):(ci + 1) * (CHE_C // 16)],
                    CHE_C, reg_c, D, single_packet=(CHE_C <= 1024))
                # xbar transpose: ghT[c, q, e] = gh[e, q*128+c], q = t*2 + gch
                ghT = pc_gt.tile([P, 2 * CHT_C, P], F16, name=f"ghT{ci}",
                                 tag="ghT")
                nc.sync.dma_start_transpose(out=ghT[:], in_=gh[:])
                sbc2 = pc_s.tile([P, CHT_C, P], U8, name=f"sbc2_{ci}", tag="sbc2")
                nc.sync.dma_start(
                    out=sbc2[:],
                    in_=slots_r_d[ci * CHT_C:(ci + 1) * CHT_C, :].partition_broadcast(P))
                STc = pc_s.tile([P, CHT_C, P], F16, name=f"STc{ci}", tag="STc")
                nc.vector.tensor_tensor(
                    out=STc[:], in0=sbc2[:],
                    in1=iota_col[:, :1, None].to_broadcast([P, CHT_C, P]),
                    op=OP.is_equal)
                # hdT[c, eh, e] = sum_slot h_blk[slot, c] * ST[slot, e]
                hd = {}
                for (b, t0, t1, new) in chunk_runs[ci]:
                    if new:
                        r = P if b < NBLK - 1 else LASTR
                        cur_hb = pc_blk.tile([P, D], F16,
                                             name=f"hb_{ci}_{t0}", tag="hb")
                        if r < P:
                            nc.vector.memset(cur_hb[:], 0.0)
                        nc.sync.dma_start(out=cur_hb[:r, :],
                                          in_=h_shard[b * P:b * P + r, :])
                    for eg in range(t0 // 4, (t1 + 3) // 4):
                        if eg not in hd:
                            hd[eg] = ps_hd.tile([P, 2, 4 * P], F32,
                                                name=f"hd{ci}_{eg}", tag="hd")
                        a = max(t0, eg * 4)
                        z = min(t1, (eg + 1) * 4)
                        for gch in range(2):
                            nc.tensor.matmul(
                                hd[eg][:, gch, (a - eg * 4) * P:(z - eg * 4) * P],
                                lhsT=cur_hb[:, gch * P:(gch + 1) * P],
                                rhs=STc[:, a:z, :],
                                start=True, stop=True)
                sc_st = pc_st.tile([3, CHT_C * P], F32, name=f"scst{ci}",
                                   tag="scst")
                for eg in range(NEG):
                    repT = pc_r.tile([P, 2, 4 * P], F16, name=f"rp{ci}_{eg}",
                                     tag="rp")
                    for gch in range(2):
                        nc.vector.tensor_tensor(
                            out=repT[:, gch, :].rearrange("p (t e) -> p t e",
                                                          e=P),
                            in0=ghT[:, eg * 8 + gch:(eg + 1) * 8:2, :],
                            in1=hd[eg][:, gch, :].rearrange("p (t e) -> p t e",
                                                            e=P),
                            op=OP.mult)
                    scT = ps_sc.tile([3, 4 * P], F32, name=f"scT{ci}_{eg}",
                                     tag="scT")
                    for gch in range(2):
                        nc.tensor.matmul(scT[:],
                                         lhsT=wk_t[:, gch, :],
                                         rhs=repT[:, gch, :],
                                         start=(gch == 0), stop=(gch == 1))
                    nc.scalar.activation(sc_st[:, eg * 4 * P:(eg + 1) * 4 * P],
                                         scT[:], AF.Copy)
                nc.sync.dma_start(
                    out=scores_t[:, ci * CHE_C:(ci + 1) * CHE_C],
                    in_=sc_st[:])

        if debug:
            tc.strict_bb_all_engine_barrier()
            nc.sync.dma_start(out=dbg_xp.ap(), in_=xp_shard.ap())
            nc.sync.dma_start(out=dbg_h.ap(), in_=h_shard.ap())

    nc.compile()
    return nc


def _host_prep(inputs, meta):
    """Build the per-core input maps from full inputs + edge metadata."""
    x = np.asarray(inputs["x"], dtype=np.float32)
    W = np.asarray(inputs["W"], dtype=np.float32)
    att_src = np.asarray(inputs["att_src"], dtype=np.float32)
    att_dst = np.asarray(inputs["att_dst"], dtype=np.float32)
    bias = np.asarray(inputs["bias"], dtype=np.float32)
    fc1_W = np.asarray(inputs["fc1_W"], dtype=np.float32)

    am_s = np.zeros((D, HEADS), dtype=np.float32)
    am_d = np.zeros((D, HEADS), dtype=np.float32)
    for h in range(HEADS):
        am_s[h * HID:(h + 1) * HID, h] = att_src[h]
        am_d[h * HID:(h + 1) * HID, h] = att_dst[h]
    w_ext = np.concatenate([W, W @ am_s, W @ am_d], axis=1)  # [256, 264]
    # wk[c, g, k] = 0.5 * fc1_W[g*128 + c, k]
    wk = np.ascontiguousarray(
        (fc1_W * 0.5).reshape(2, P, 3).transpose(1, 0, 2)).astype(np.float16)
    iota_col_v = np.arange(P, dtype=np.uint8)[:, None]
    iota_row8 = np.broadcast_to(np.arange(P, dtype=np.uint8)[None, :],
                                (P, P)).copy()
    bias_bc = np.broadcast_to(bias[None, :], (P, D)).copy()
    eye = np.eye(P, dtype=np.float32)

    in_maps = []
    for c in range(NCORE):
        pc = meta["per_core"][c]
        in_maps.append({
            "x_shard": np.ascontiguousarray(x[c * NPC:(c + 1) * NPC]),
            "w0e": np.ascontiguousarray(w_ext[0:P]),
            "w1e": np.ascontiguousarray(w_ext[P:2 * P]),
            "wk_d": wk,
            "bias_bc": bias_bc,
            "eye_d": eye,
            "idx_d": pc["idx_w"],
            "slots_d": np.minimum(pc["slots"], 255).astype(np.uint8),
            "slots_r_d": np.ascontiguousarray(
                np.minimum(pc["slots"].T, 255)).astype(np.uint8),
            "iota_col_d": iota_col_v,
            "iota_row8_d": iota_row8,
        })
    return in_maps


def _assemble(meta, results, fc1_b):
    out = np.zeros((E, 3), dtype=np.float32)
    for c in range(NCORE):
        raw = results[c]["scores_t"]  # [3, TOTE]
        sc = np.asarray(raw, dtype=np.float32).T  # [TOTE, 3]
        orig = meta["per_core"][c]["orig"]
        m = orig >= 0
        out[orig[m]] = sc[m]
    out += np.asarray(fc1_b, dtype=np.float32)[None, :]
    return out


def kernel(**inputs):
    edge_index = np.asarray(inputs["edge_index"])
    fc1_b = np.asarray(inputs["fc1_b"], dtype=np.float32)

    meta = _preprocess(edge_index)

    trace = os.environ.get("KERNEL_TRACE", "0") == "1"
    if trace:
        import concourse.bass_utils as bass_utils
        try:
            from trn_agent_boot.trn_boot import _ntff_profile_via_ctypes
            mod = types.ModuleType("antenv.axon_hooks")
            hook = _ntff_profile_via_ctypes("/opt/axon/libaxon_pjrt.so")
            mod.get_axon_ntff_profile_hook = lambda: hook
            mod.set_axon_ntff_profile_hook = lambda h: None
            sys.modules["antenv.axon_hooks"] = mod
            bass_utils.upload_artifacts = lambda tmpdir: f"local:{tmpdir}"
        except Exception as e:  # profiling optional
            print("trace hook setup failed:", e)
            trace = False

    nc = _build(meta)

    from concourse.bass_utils import run_bass_kernel_spmd

    in_maps = _host_prep(inputs, meta)

    res = run_bass_kernel_spmd(nc, in_maps, list(range(NCORE)), trace=trace)
    global _last_results
    _last_results = res
    if trace and res.exec_time_ns:
        print(f"HW exec time: {res.exec_time_ns} ns")

    return _assemble(meta, [res.results[c] for c in range(NCORE)], fc1_b)


if __name__ == "__main__":
    import reference
    inputs = reference.setup_inputs()
    inputs = {k: np.asarray(v) for k, v in inputs.items()}
    got = kernel(**inputs)
    exp = np.asarray(reference.reference(**{k: v for k, v in inputs.items()}))
    denom = np.abs(exp).max()
    rel = np.abs(got - exp).max() / denom
    print("Relative error:", rel)
